# revision 5
# baseline (speedup 1.0000x reference)
"""Causal self-attention Bass/Trainium2 kernel.

Problem: B=4, T=2048, D=1024, H=16, DH=64 (fp32 reference).
Sharding: 8 cores = 4 batches x 2 head-groups (DP x TP). Each core computes
a [2048, 1024] partial of out = attn(x[b]) @ Wout for its 8 heads; the host
sums the two TP partials per batch.

Per-core layout strategy (all "T" means transposed so the tensor engine's
[K-partition, N-free] operand shapes line up with no on-chip transposes):
  - qkvT projection computes qT/kT = Wqkv_blk.T @ xT directly in [dh, t]
    layout (fp32r matmuls); V is computed in natural [t, dh] layout by
    swapping stationary/moving operands.
  - scoresT[k, q] = K @ Q.T per head (fp32r, K=64 contraction), causal
    suffix only.
  - softmax: exp on ACT (scale=1/8 folded in), no max-subtraction (scores
    are O(1) by construction), denominators via an extra ones-column in the
    attn@V stationary operand; normalization deferred to a per-column
    multiply using gpsimd partition_broadcast of the reciprocals.
  - attn@V accumulates outT[dh+1, q] in PSUM over k-tiles (bf16).
  - Wout projection: lhsT = normalized attn_outT (bf16), accumulated over
    the 4 head-pair tiles, written straight from PSUM to DRAM.
"""
import sys

if "/opt/trn_rl_repo" not in sys.path:
    sys.path.insert(0, "/opt/trn_rl_repo")

import numpy as np

B, T, D, H, DH = 4, 2048, 1024, 16, 64
NCORES = 8
HL = 8          # heads per core
PAIRS = 4       # head pairs per core (2 heads share a 128-partition tile)
DKT = 8         # D/128 contraction tiles
TT128 = 16      # T/128 tiles
TC512 = 4       # T/512 chunks
QC = 2          # q chunks of 1024 per head
KT = 16         # k 128-tiles

_NC = None


def _build_nc():
    import concourse.bass as bass
    from concourse import bacc
    import concourse.mybir as mybir
    from concourse.tile import TileContext

    f = mybir.dt.float32
    r = mybir.dt.float32r
    bf = mybir.dt.bfloat16
    EXPF = mybir.ActivationFunctionType.Exp

    nc = bacc.Bacc("TRN2", target_bir_lowering=False)
    xT = nc.declare_dram_parameter("xT", [D, T], r, isOutput=False)
    wqk = nc.declare_dram_parameter("wqk", [D, 1024], r, isOutput=False)
    wv = nc.declare_dram_parameter("wv", [D, 512], r, isOutput=False)
    wout = nc.declare_dram_parameter("wout", [512, 1024], bf, isOutput=False)
    cos2 = nc.declare_dram_parameter("cos2", [128, T], f, isOutput=False)
    sin2s = nc.declare_dram_parameter("sin2s", [128, T], f, isOutput=False)
    umask = nc.declare_dram_parameter("umask", [128, 128], bf, isOutput=False)
    out = nc.declare_dram_parameter("out", [T, D], f, isOutput=True)

    with TileContext(nc) as tc:
        with tc.tile_pool(name="psum", bufs=1, space="PSUM") as pp, \
             tc.tile_pool(name="main", bufs=1) as pool:
            # ---- persistent SBUF tiles -------------------------------------
            vsb = []  # V_sb[tt]: [128, 520] bf16, col 65h+64 is the ones col
            for tt in range(TT128):
                vsb.append(pool.tile([128, 8 * 65], bf, tag="vsb", bufs=TT128, name=f"vsb{tt}"))
            aoT = []  # normalized attn-out^T per pair: [128, T] bf16
            for p in range(PAIRS):
                aoT.append(pool.tile([128, T], bf, tag="aoT", bufs=PAIRS, name=f"aoT{p}"))
            umask_sb = pool.tile([128, 128], bf, tag="umask")
            nc.sync.dma_start(out=umask_sb[:, :], in_=umask[:, :])

            with tc.tile_pool(name="phasea", bufs=1) as ap:
                xts = []
                for dk in range(DKT):
                    xts.append(ap.tile([128, T], r, tag="xt", bufs=DKT, name=f"xt{dk}"))
                    nc.sync.dma_start(out=xts[dk][:, :],
                                      in_=xT[128 * dk:128 * (dk + 1), :])
                cos_sb = ap.tile([128, T], f, tag="cos_sb")
                sin_sb = ap.tile([128, T], f, tag="sin_sb")
                nc.sync.dma_start(out=cos_sb[:, :], in_=cos2[:, :])
                nc.sync.dma_start(out=sin_sb[:, :], in_=sin2s[:, :])

                # V: for each 128-row t-tile, out[t, 512] = x[t,:] @ Wv
                with tc.tile_pool(name="wvp", bufs=1) as wvp:
                    wvs = []
                    for dk in range(DKT):
                        wvs.append(wvp.tile([128, 512], r, tag="wv", bufs=DKT, name=f"wv{dk}"))
                        nc.sync.dma_start(out=wvs[dk][:, :],
                                          in_=wv[128 * dk:128 * (dk + 1), :])
                    for tt in range(TT128):
                        pv = pp.tile([128, 512], f, tag="pqk", bufs=2, name=f"pv{tt}")
                        for dk in range(DKT):
                            nc.tensor.matmul(
                                pv[:, :],
                                xts[dk][:, 128 * tt:128 * (tt + 1)],
                                wvs[dk][:, :],
                                start=(dk == 0), stop=(dk == DKT - 1),
                                skip_group_check=True)
                        dst = vsb[tt][:, :].rearrange("p (h c) -> p h c", c=65)
                        src_ = pv[:, :].rearrange("p (h c) -> p h c", c=64)
                        nc.vector.tensor_copy(dst[:, :, 0:64], src_[:, :, :])
                        nc.vector.memset(dst[:, :, 64:65], 1.0)

                # per pair: qkT + RoPE, then attention for its two heads
                bpo_cm = tc.tile_pool(name="attn", bufs=1)
                bpo = bpo_cm.__enter__()
                for p in range(PAIRS):
                    qTp = ap.tile([128, T], r, tag="qT", bufs=2, name=f"qT{p}")
                    kTp = ap.tile([128, T], r, tag="kT", bufs=2, name=f"kT{p}")
                    for qk in range(2):  # 0 = q, 1 = k
                        dstT = qTp if qk == 0 else kTp
                        col0 = 512 * qk + 128 * p
                        wblks = []
                        for dk in range(DKT):
                            wb = ap.tile([128, 128], r, tag="wqk", bufs=2 * DKT, name=f"wqk{p}_{qk}_{dk}")
                            nc.sync.dma_start(
                                out=wb[:, :],
                                in_=wqk[128 * dk:128 * (dk + 1), col0:col0 + 128])
                            wblks.append(wb)
                        for tcii in range(TC512):
                            ts0 = 512 * tcii
                            pq = pp.tile([128, 512], f, tag="pqk", bufs=2, name=f"pq{p}_{qk}_{tcii}")
                            for dk in range(DKT):
                                nc.tensor.matmul(
                                    pq[:, :], wblks[dk][:, :],
                                    xts[dk][:, ts0:ts0 + 512],
                                    start=(dk == 0), stop=(dk == DKT - 1),
                                    skip_group_check=True)
                            # RoPE: out = pq*cos + swap32(pq)*sin_signed
                            tmpa = ap.tile([128, 512], f, tag="ropeA", bufs=1, name=f"ra{p}_{qk}_{tcii}")
                            nc.vector.tensor_mul(tmpa[:, :], pq[:, :],
                                                 cos_sb[:, ts0:ts0 + 512])
                            pcp = ap.tile([128, 512], f, tag="ropeP", bufs=1, name=f"rp{p}_{qk}_{tcii}")
                            nc.scalar.copy(pcp[:, :], pq[:, :])
                            sw = ap.tile([128, 512], f, tag="ropeB", bufs=1, name=f"rb{p}_{qk}_{tcii}")
                            for qtr in range(4):
                                d0 = 32 * qtr
                                s0 = 32 * (qtr + 1) if qtr % 2 == 0 else 32 * (qtr - 1)
                                nc.sync.dma_start(
                                    out=sw[d0:d0 + 32, :],
                                    in_=pcp[s0:s0 + 32, :])
                            tmpc = ap.tile([128, 512], f, tag="ropeC", bufs=1, name=f"rc{p}_{qk}_{tcii}")
                            nc.vector.tensor_mul(tmpc[:, :], sw[:, :],
                                                 sin_sb[:, ts0:ts0 + 512])
                            nc.vector.tensor_add(dstT[:, ts0:ts0 + 512],
                                                 tmpa[:, :], tmpc[:, :])

                    # ---- attention for the two heads of this pair ----------
                    for i in range(2):
                        h = 2 * p + i
                        hb = 64 * i
                        for c in range(QC):
                            q0 = 1024 * c
                            acc = pp.tile([65, 1024], f, tag="accum", bufs=1, name=f"acc{h}_{c}")
                            ktmax = 8 * c + 7
                            for kt in range(ktmax + 1):
                                qlo = max(q0, 128 * kt)      # global q start
                                off = qlo - q0               # offset in chunk
                                sc = pp.tile([128, 1024], f, tag="scores", bufs=2, name=f"sc{h}_{c}_{kt}")
                                o = off
                                while o < 1024:
                                    sw_ = min(512, ((o // 512) + 1) * 512 - o,
                                              1024 - o)
                                    nc.tensor.matmul(
                                        sc[:, o:o + sw_],
                                        kTp[hb:hb + 64,
                                            128 * kt:128 * (kt + 1)],
                                        qTp[hb:hb + 64,
                                            q0 + o:q0 + o + sw_],
                                        start=True, stop=True,
                                        skip_group_check=True)
                                    o += sw_
                                et = bpo.tile([128, 1024], bf, tag="expT", bufs=2, name=f"et{h}_{c}_{kt}")
                                nc.scalar.activation(
                                    et[:, off:1024], sc[:, off:1024], EXPF,
                                    scale=0.125)
                                if 128 * kt >= q0:  # diagonal block: mask
                                    nc.vector.tensor_mul(
                                        et[:, off:off + 128],
                                        et[:, off:off + 128],
                                        umask_sb[:, :])
                                o = off
                                while o < 1024:
                                    sw_ = min(512, ((o // 512) + 1) * 512 - o,
                                              1024 - o)
                                    nc.tensor.matmul(
                                        acc[:, o:o + sw_],
                                        vsb[kt][:, 65 * h:65 * h + 65],
                                        et[:, o:o + sw_],
                                        start=(kt == 0), stop=(kt == ktmax),
                                        skip_group_check=True)
                                    o += sw_
                            # normalization for this (head, chunk)
                            drow = bpo.tile([1, 1024], f, tag="drow", bufs=1, name=f"drow{h}_{c}")
                            nc.vector.tensor_copy(drow[:, :], acc[64:65, :])
                            deng = bpo.tile([8, 128], f, tag="deng", bufs=1, name=f"deng{h}_{c}")
                            nc.sync.dma_start(out=deng[:, :], in_=drow[:, :])
                            rec8 = bpo.tile([8, 128], f, tag="rec8", bufs=1, name=f"rec8_{h}_{c}")
                            nc.vector.reciprocal(rec8[:, :], deng[:, :])
                            rec1 = bpo.tile([1, 1024], f, tag="rec1", bufs=1, name=f"rec1_{h}_{c}")
                            nc.sync.dma_start(out=rec1[:, :], in_=rec8[:, :])
                            bca = bpo.tile([64, 1024], f, tag="bca", bufs=1, name=f"bca{h}_{c}")
                            nc.gpsimd.partition_broadcast(bca[:, :], rec1[0:1, :])
                            if i == 0:
                                nc.vector.tensor_mul(
                                    aoT[p][0:64, q0:q0 + 1024],
                                    acc[0:64, :], bca[:, :])
                            else:
                                tb = bpo.tile([64, 1024], bf, tag="tmpb", bufs=1, name=f"tb{h}_{c}")
                                nc.vector.tensor_mul(tb[:, :], acc[0:64, :],
                                                     bca[:, :])
                                nc.sync.dma_start(
                                    out=aoT[p][64:128, q0:q0 + 1024],
                                    in_=tb[:, :])

                bpo_cm.__exit__(None, None, None)

            # ---- phase C: output projection --------------------------------
            with tc.tile_pool(name="phasec", bufs=1) as cp:
                wouts = []
                for p in range(PAIRS):
                    wouts.append(cp.tile([128, 1024], bf, tag="wout", bufs=PAIRS, name=f"wo{p}"))
                    nc.sync.dma_start(out=wouts[p][:, :],
                                      in_=wout[128 * p:128 * (p + 1), :])
                for tt in range(TT128):
                    for e in range(2):
                        po = pp.tile([128, 512], f, tag="pqk", bufs=2, name=f"po{tt}_{e}")
                        for p in range(PAIRS):
                            nc.tensor.matmul(
                                po[:, :],
                                aoT[p][:, 128 * tt:128 * (tt + 1)],
                                wouts[p][:, 512 * e:512 * (e + 1)],
                                start=(p == 0), stop=(p == PAIRS - 1),
                                skip_group_check=True)
                        ob = cp.tile([128, 512], f, tag="obuf", bufs=3, name=f"ob{tt}_{e}")
                        nc.scalar.copy(ob[:, :], po[:, :])
                        nc.sync.dma_start(
                            out=out[128 * tt:128 * (tt + 1),
                                    512 * e:512 * (e + 1)],
                            in_=ob[:, :])

    nc.compile()
    return nc


def _get_nc():
    global _NC
    if _NC is None:
        _NC = _build_nc()
    return _NC


def _host_prep(x, cos, sin, Wqkv, Wout):
    """Build the 8 per-core input maps."""
    cosf = np.asarray(cos, dtype=np.float32)[0, :, 0, :]   # [T, DH]
    sinf = np.asarray(sin, dtype=np.float32)[0, :, 0, :]
    cos_half = np.ascontiguousarray(cosf[:, :32].T)        # [32, T]
    sin_half = np.ascontiguousarray(sinf[:, :32].T)
    cos2 = np.tile(cos_half, (4, 1)).astype(np.float32)
    sin2s = np.concatenate([-sin_half, sin_half, -sin_half, sin_half],
                           axis=0).astype(np.float32)
    import ml_dtypes
    umask = np.triu(np.ones((128, 128), dtype=np.float32)).astype(
        ml_dtypes.bfloat16)

    Wq = Wqkv[:, 0:D]
    Wk = Wqkv[:, D:2 * D]
    Wv = Wqkv[:, 2 * D:3 * D]

    in_maps = []
    for core in range(NCORES):
        b, hg = core // 2, core % 2
        h0 = HL * hg
        xTb = np.ascontiguousarray(x[b].T).astype(np.float32)
        wqk_c = np.ascontiguousarray(np.concatenate(
            [Wq[:, DH * h0:DH * (h0 + HL)], Wk[:, DH * h0:DH * (h0 + HL)]],
            axis=1)).astype(np.float32)
        wv_c = np.ascontiguousarray(Wv[:, DH * h0:DH * (h0 + HL)]).astype(
            np.float32)
        wout_c = np.ascontiguousarray(
            Wout[512 * hg:512 * (hg + 1), :]).astype(ml_dtypes.bfloat16)
        in_maps.append({
            "xT": xTb, "wqk": wqk_c, "wv": wv_c, "wout": wout_c,
            "cos2": cos2, "sin2s": sin2s, "umask": umask,
        })
    return in_maps


def kernel(x, cos, sin, Wqkv, Wout):
    from concourse.bass_utils import run_bass_kernel_spmd

    nc = _get_nc()
    in_maps = _host_prep(np.asarray(x, dtype=np.float32), cos, sin,
                         np.asarray(Wqkv, dtype=np.float32),
                         np.asarray(Wout, dtype=np.float32))
    res = run_bass_kernel_spmd(nc, in_maps, core_ids=list(range(NCORES)))
    outp = np.empty((B, T, D), dtype=np.float32)
    for b in range(B):
        outp[b] = res.results[2 * b]["out"] + res.results[2 * b + 1]["out"]
    return outp
